# revision 10
# baseline (speedup 1.0000x reference)
"""Trainium2 Bass kernel for nn_LogicDense (difflogic dense layer).

Math (reference):
    w      = softmax(weight, axis=-1)            # [out_dim, 16]
    coeffs = w @ GATE_COEFFS                     # [out_dim, 4] = (c0, ca, cb, cab)
    a      = x[:, indices[0]]                    # [batch, out_dim]
    b      = x[:, indices[1]]
    out    = c0 + ca*a + cb*b + cab*a*b          # [batch, out_dim]

Strategy (8 NeuronCores, tensor-parallel over out_dim):
    - All 16 gates map [0,1]^2 -> [0,1] and softmax weights are convex, so
      out is in [0,1]; x is uniform [0,1). 8-bit fixed point fits the 2e-2
      rel-err budget on both ends:
        * the a-side gather reads sa = round(250*x) uint8 from HBM (4 KB
          rows); it is consumed DIRECTLY (no cast): tensor_scalar accepts
          the u8 operand at 2x_2P and ACT converts on read anyway;
        * the output is computed as out_u = 250*out + 2.5 in fp16 and the
          store DMA (SWDGE, gpsimd) casts fp16 -> uint8 on the way out.
      Factorization keeps the fp16 x fp16 TT mul pure:
        out_u = b*(250*cb + cab*sa) + (ca*sa + 250*c0+2.5)
        t = (sa * cab) + 250*cb      (DVE tensor_scalar, u8 src)
        h = ca*sa + (250*c0+2.5)     (ACT Identity, scale/bias APs, u8 src)
        g = t * b                    (DVE tensor_mul, fp16)
        o = g + h                    (DVE tensor_add) in [2, 253]
    - HBM/core: 8 MiB a (u8) + 16 MiB b (fp16) + 8 MiB store (u8) = 32 MiB.
    - Gathers run at 256-index granularity (two compute chunks per call)
      to amortize the ~1.2us fixed Q7 ucode cost per dma_gather; compute
      stays per-128-row chunk (slot s of the gather dst) so the
      per-partition coeff-scalar trick keeps working.
    - Softmax+gate-coeff collapse on device as before; the 250/2.5 scaling
      lives in the host gcr constant (softmax weights sum to 1, so the
      offset rides the convex combination exactly).
    - Host concatenates the 8 uint8 shards, transposes, decodes
      (u - 2.5)/250 in fp32.
"""

import os
import sys

import numpy as np

sys.path.insert(0, "/opt/trn_rl_repo")

BATCH = 4096
IN_DIM = 8192
OUT_DIM = 16384
N_CORES = 8
J_SHARD = OUT_DIM // N_CORES        # 2048 output rows per core
CHUNK = 128                         # output rows per pipeline iteration
N_CHUNKS = J_SHARD // CHUNK         # 16
GCHUNK = 256                        # output rows per dma_gather call
N_GCH = J_SHARD // GCHUNK           # 8 gather iterations
NG = 3                              # gather buffer sets (a and b each)
NT = 4                              # t buffer sets (ts -> mul lifetime)
NH = 4                              # h buffer sets (h -> add lifetime)
NO = 3                              # paired output buffer sets
DVE_PRE = 14                        # DVE preamble (coeff) instruction count

OUT_SCALE = 250.0                   # out_u = OUT_SCALE*out + OUT_OFF
OUT_OFF = 2.5

GATE_COEFFS = np.array([
    [0, 0, 0, 0], [0, 0, 0, 1], [0, 1, 0, -1], [0, 1, 0, 0],
    [0, 0, 1, -1], [0, 0, 1, 0], [0, 1, 1, -2], [0, 1, 1, -1],
    [1, -1, -1, 1], [1, -1, -1, 2], [1, 0, -1, 0], [1, 0, -1, 1],
    [1, -1, 0, 0], [1, -1, 0, 1], [1, 0, 0, -1], [1, 0, 0, 0],
], dtype=np.float32)                # [16 gates, 4 bilinear coeffs]

_CACHE = {}
LAST_RESULT = None  # BassKernelResults of the most recent run (for profiling)


def _wrap_idx256(grp):
    """Wrap one 256-index gather list into dma_gather's index layout:
    index j lives at [j%16, j//16] of a [16, 16] block, replicated across
    the 8 groups of 16 partitions (the Q7 tx/rx cpus read the indices
    from different partition groups). Returns [128, 16] int16."""
    blk = grp.astype(np.int16).reshape(16, 16).T    # [16, 16]
    return np.tile(blk, (8, 1))                     # [128, 16]


def _wrap_core_idx(idx_pair):
    """Per-core [2, J_SHARD] -> [128, 32*N_GCH] int16; gather iter P uses
    cols [32P, 32P+16) for the a-gather (idx0 of outputs [256P, 256P+256))
    and [32P+16, 32P+32) for the b-gather (idx1)."""
    cols = []
    for c in range(N_GCH):
        s = slice(c * GCHUNK, (c + 1) * GCHUNK)
        cols.append(_wrap_idx256(idx_pair[0, s]))
        cols.append(_wrap_idx256(idx_pair[1, s]))
    return np.ascontiguousarray(np.concatenate(cols, axis=1))


def _build_program():
    import concourse.bacc as bacc
    import concourse.mybir as mybir
    from concourse.library_config import mlp
    from contextlib import ExitStack

    dt = mybir.dt
    AF = mybir.ActivationFunctionType

    nc = bacc.Bacc("TRN2", target_bir_lowering=False, debug=False)

    xt16 = nc.dram_tensor("xt16", [IN_DIM, BATCH], dt.float16,
                          kind="ExternalInput")
    xt8 = nc.dram_tensor("xt8", [IN_DIM, BATCH], dt.uint8,
                         kind="ExternalInput")
    idx = nc.dram_tensor("idx", [128, 32 * N_GCH], dt.int16,
                         kind="ExternalInput")
    wgt = nc.dram_tensor("wgt", [128, N_CHUNKS * 16], dt.float32,
                         kind="ExternalInput")
    gcr = nc.dram_tensor("gcr", [128, 4 * N_CHUNKS * 16], dt.float32,
                         kind="ExternalInput")
    out = nc.dram_tensor("out", [J_SHARD, BATCH], dt.uint8,
                         kind="ExternalOutput")

    W16 = N_CHUNKS * 16  # 256: free size of the wrapped weight / exp tiles

    with ExitStack() as ctx:
        sb = lambda name, shape, dty: ctx.enter_context(
            nc.sbuf_tensor(name, shape, dty))
        sb_idx = sb("sb_idx", [128, 32 * N_GCH], dt.int16)
        sb_w = sb("sb_w", [128, W16], dt.float32)
        sb_gc = sb("sb_gc", [128, 4 * W16], dt.float32)
        sb_e = sb("sb_e", [128, W16], dt.float32)
        sb_scr = sb("sb_scr", [128, W16], dt.float32)
        sb_s = sb("sb_s", [128, N_CHUNKS], dt.float32)
        sb_r = sb("sb_r", [128, N_CHUNKS], dt.float32)
        # coeff tile: [:, 16*k + c] = coeff k (0=c0,1=ca,2=cb,3=cab), chunk c
        sb_cc = sb("sb_cc", [128, 4 * N_CHUNKS], dt.float32)
        # gather dst layout: [128, 2, BATCH] -- slot s holds compute chunk
        # 2P+s of gather iter P (index j of the 256-list lands on partition
        # j%128, slot j//128).
        a_bufs = [sb(f"a{k}", [128, 2, BATCH], dt.uint8) for k in range(NG)]
        b_bufs = [sb(f"b{k}", [128, 2, BATCH], dt.float16) for k in range(NG)]
        t_bufs = [sb(f"t{k}", [128, BATCH], dt.float16) for k in range(NT)]
        h_bufs = [sb(f"h{k}", [128, BATCH], dt.float16) for k in range(NH)]
        o_bufs = [sb(f"o{k}", [128, 2, BATCH], dt.float16) for k in range(NO)]

        # ts(i): t = (sa*cab) + 250*cb. It is ACT-shaped (scale/bias per
        # partition), so alternate chunks compute it on ACT (even i) vs DVE
        # (odd i) to balance the two engines:
        #   DVE: 8 ts (2.4us) + 16 mul + 16 add (2.3us)  ~= 92us
        #   ACT: 8 t (3.6us) + 16 h (3.6us) + exp        ~= 88us
        ts_on_act = lambda i: i % 2 == 0

        ops_act = []
        for i in range(N_CHUNKS):
            if ts_on_act(i):
                ops_act.append(('t', i))
            ops_act.append(('h', i))
        act_val = {op: n + 1 for n, op in enumerate(ops_act)}

        ops_dve = []  # DVE stream after the coeff preamble
        for i in range(N_CHUNKS):
            if not ts_on_act(i):
                ops_dve.append(('ts', i))
            if i > 0:
                ops_dve.append(('add', i - 1))
            ops_dve.append(('mul', i))
        ops_dve.append(('add', N_CHUNKS - 1))
        dve_val = {op: DVE_PRE + n + 1 for n, op in enumerate(ops_dve)}

        with (
            nc.Block() as block,
            nc.semaphore("s_pi") as s_pi,
            nc.semaphore("s_pw") as s_pw,
            nc.semaphore("s_pg") as s_pg,
            nc.semaphore("s_exp") as s_exp,
            nc.semaphore("s_ga0") as s_ga0,
            nc.semaphore("s_ga1") as s_ga1,
            nc.semaphore("s_ga2") as s_ga2,
            nc.semaphore("s_gb0") as s_gb0,
            nc.semaphore("s_gb1") as s_gb1,
            nc.semaphore("s_gb2") as s_gb2,
            nc.semaphore("s_st0") as s_st0,
            nc.semaphore("s_st1") as s_st1,
            nc.semaphore("s_st2") as s_st2,
            nc.semaphore("s_act") as s_act,
            nc.semaphore("s_dve") as s_dve,
        ):
            s_ga = [s_ga0, s_ga1, s_ga2]
            s_gb = [s_gb0, s_gb1, s_gb2]
            s_st = [s_st0, s_st1, s_st2]

            def cseg(k, i):  # per-partition scalar AP: coeff k, chunk i
                return sb_cc[:, 16 * k + i : 16 * k + i + 1]

            @block.sync
            def _(sync):
                sync.dma_start(sb_idx[:, :], idx[:, :]).then_inc(s_pi, 16)
                sync.dma_start(sb_w[:, :], wgt[:, :]).then_inc(s_pw, 16)
                sync.dma_start(sb_gc[:, :], gcr[:, :]).then_inc(s_pg, 16)

            @block.gpsimd
            def _(gp):
                gp.load_library(mlp)
                nreg = gp.alloc_register("nidx")
                gp.reg_mov(nreg, GCHUNK)
                gp.wait_ge(s_pi, 16)  # idx tile loaded

                def store(jp):  # store chunk pair jp (chunks 2jp, 2jp+1)
                    ko = jp % NO
                    gp.wait_ge(s_dve, dve_val[('add', 2 * jp + 1)])
                    if jp >= NO:
                        gp.wait_ge(s_st[ko], 16 * (jp // NO))
                    dst = out[jp * 2 * CHUNK:(jp + 1) * 2 * CHUNK, :]
                    gp.dma_start(dst.rearrange("(s p) f -> p s f", p=CHUNK),
                                 o_bufs[ko][:, :, :]).then_inc(s_st[ko], 16)

                next_store = 0

                for P in range(N_GCH):
                    kg = P % NG
                    last = 2 * (P - NG) + 1
                    if P >= NG:
                        # a[kg] free once h (ACT) + ts of its last compute
                        # chunk are done; mul(last) implies ts(last) and
                        # frees b[kg] as well.
                        gp.wait_ge(s_act, act_val[('h', last)])
                        gp.wait_ge(s_dve, dve_val[('mul', last)])
                    gp.dma_gather(
                        a_bufs[kg].ap(), xt8.ap(),
                        sb_idx[:, 32 * P:32 * P + 16], GCHUNK, nreg, BATCH,
                    ).then_inc(s_ga[kg], 16)
                    gp.dma_gather(
                        b_bufs[kg].ap(), xt16.ap(),
                        sb_idx[:, 32 * P + 16:32 * P + 32], GCHUNK, nreg,
                        BATCH,
                    ).then_inc(s_gb[kg], 16)
                    # one paired store per iter, lagging compute
                    if P >= 2:
                        for jp in range(next_store, P - 1):
                            store(jp)
                        next_store = P - 1
                for jp in range(next_store, N_CHUNKS // 2):
                    store(jp)
                for ko in range(NO):
                    n_st = (N_CHUNKS // 2 - 1 - ko) // NO + 1
                    gp.wait_ge(s_st[ko], 16 * n_st)

            @block.scalar
            def _(sc):
                sc.wait_ge(s_pw, 16)
                sc.activation(sb_e[:, :], sb_w[:, :], AF.Exp).then_inc(s_exp, 1)
                sc.wait_ge(s_dve, DVE_PRE)  # coeff tile ready
                for kind, i in ops_act:
                    kg = (i // 2) % NG
                    sc.wait_ge(s_ga[kg], 16 * (i // 2 // NG + 1))
                    if kind == 't':
                        kt = i % NT
                        # t[kt] free once DVE mul of i-NT consumed it
                        if i >= NT:
                            sc.wait_ge(s_dve, dve_val[('mul', i - NT)])
                        # t = cab*sa + 250*cb  (u8 source read directly)
                        sc.activation(t_bufs[kt][:, :], a_bufs[kg][:, i % 2, :],
                                      AF.Identity,
                                      bias=cseg(2, i), scale=cseg(3, i),
                                      ).then_inc(s_act, 1)
                    else:
                        kh = i % NH
                        # h[kh] free once DVE add of i-NH completed
                        if i >= NH:
                            sc.wait_ge(s_dve, dve_val[('add', i - NH)])
                        # h = ca*sa + (250*c0 + 2.5)
                        sc.activation(h_bufs[kh][:, :], a_bufs[kg][:, i % 2, :],
                                      AF.Identity,
                                      bias=cseg(0, i), scale=cseg(1, i),
                                      ).then_inc(s_act, 1)

            @block.vector
            def _(v):
                X = mybir.AxisListType.X
                n = 0

                def step(ins):
                    nonlocal n
                    n += 1
                    ins.then_inc(s_dve, 1)

                v.wait_ge(s_exp, 1)
                v.wait_ge(s_pg, 16)  # gc tile loaded
                e3 = sb_e[:, :].rearrange("p (c g) -> p c g", g=16)
                step(v.reduce_sum(sb_s[:, :], e3, axis=X))
                step(v.reciprocal(sb_r[:, :], sb_s[:, :]))
                for kk in range(4):
                    step(v.tensor_mul(sb_scr[:, :], sb_e[:, :],
                                      sb_gc[:, kk * W16:(kk + 1) * W16]))
                    step(v.reduce_sum(
                        sb_cc[:, 16 * kk:16 * (kk + 1)],
                        sb_scr[:, :].rearrange("p (c g) -> p c g", g=16),
                        axis=X))
                for kk in range(4):
                    step(v.tensor_mul(sb_cc[:, 16 * kk:16 * (kk + 1)],
                                      sb_cc[:, 16 * kk:16 * (kk + 1)],
                                      sb_r[:, :]))
                assert n == DVE_PRE
                MU, AD = mybir.AluOpType.mult, mybir.AluOpType.add
                for kind, i in ops_dve:
                    kt, kh = i % NT, i % NH
                    ko = (i // 2) % NO
                    kg = (i // 2) % NG
                    if kind == 'ts':
                        # t = (sa * cab) + 250*cb  (u8 src, 2x_2P mode)
                        v.wait_ge(s_ga[kg], 16 * (i // 2 // NG + 1))
                        v.tensor_scalar(t_bufs[kt][:, :],
                                        a_bufs[kg][:, i % 2, :],
                                        cseg(3, i), cseg(2, i), MU, AD,
                                        ).then_inc(s_dve, 1)
                    elif kind == 'mul':
                        v.wait_ge(s_gb[kg], 16 * (i // 2 // NG + 1))
                        if ts_on_act(i):
                            v.wait_ge(s_act, act_val[('t', i)])
                        if i // 2 >= NO:
                            # o[ko] free once store of pair i//2-NO completed
                            v.wait_ge(s_st[ko], 16 * (i // 2 // NO))
                        v.tensor_mul(o_bufs[ko][:, i % 2, :], t_bufs[kt][:, :],
                                     b_bufs[kg][:, i % 2, :]).then_inc(s_dve, 1)
                    else:  # add
                        v.wait_ge(s_act, act_val[('h', i)])
                        v.tensor_add(o_bufs[ko][:, i % 2, :],
                                     o_bufs[ko][:, i % 2, :],
                                     h_bufs[kh][:, :]).then_inc(s_dve, 1)

    nc.compile()
    return nc


def _get_program():
    if "nc" not in _CACHE:
        _CACHE["nc"] = _build_program()
    return _CACHE["nc"]


def kernel(x, weight, indices):
    global LAST_RESULT
    from concourse.bass_utils import run_bass_kernel_spmd

    x = np.asarray(x, dtype=np.float32)
    weight = np.asarray(weight, dtype=np.float32)
    indices = np.asarray(indices)

    nc = _get_program()

    xt = np.ascontiguousarray(x.T)                       # [in_dim, batch] f32
    xt16 = xt.astype(np.float16)
    xt8 = np.rint(xt * OUT_SCALE).astype(np.uint8)       # 250*x in [0, 250]

    # gc replicate: [p, kk*256 + 16*c + g] = scaled GATE_COEFFS[g, kk].
    # c0 row carries the 250x output scale and +2.5 offset (softmax weights
    # sum to 1, so the offset survives the convex combination exactly);
    # cb row carries the 250x scale; ca/cab are unscaled because sa = 250*x.
    gc_scaled = GATE_COEFFS.copy()
    gc_scaled[:, 0] = OUT_SCALE * gc_scaled[:, 0] + OUT_OFF
    gc_scaled[:, 2] = OUT_SCALE * gc_scaled[:, 2]
    gc_rep = np.broadcast_to(
        gc_scaled.T.reshape(4, 1, 16),                   # [kk, 1, g]
        (4, N_CHUNKS, 16)).reshape(1, -1)
    gc_rep = np.ascontiguousarray(
        np.broadcast_to(gc_rep, (128, 4 * N_CHUNKS * 16)).astype(np.float32))

    in_maps = []
    for c in range(N_CORES):
        j0 = c * J_SHARD
        idx_c = _wrap_core_idx(indices[:, j0:j0 + J_SHARD])
        wsh = weight[j0:j0 + J_SHARD]                    # [2048, 16]
        w_wrapped = np.ascontiguousarray(
            wsh.reshape(N_CHUNKS, 128, 16).transpose(1, 0, 2)
            .reshape(128, N_CHUNKS * 16))
        in_maps.append({
            "xt16": xt16,
            "xt8": xt8,
            "idx": idx_c,
            "wgt": w_wrapped,
            "gcr": gc_rep,
        })

    trace = bool(os.environ.get("KERNEL_TRACE"))
    res = run_bass_kernel_spmd(nc, in_maps, core_ids=list(range(N_CORES)),
                               trace=trace)
    LAST_RESULT = res

    shards = [res.results[c]["out"] for c in range(N_CORES)]
    full = np.concatenate(shards, axis=0)                # [out_dim, batch] u8
    dec = (full.T.astype(np.float32) - OUT_OFF) * (1.0 / OUT_SCALE)
    return np.ascontiguousarray(dec)
